# revision 8
# baseline (speedup 1.0000x reference)
"""Trainium2 Bass kernel for nn_ContrastiveLoss — PE-centric v2.

Strategy
--------
Data-parallel over batch B=512: 8 cores x 64 samples, fully independent
(only the cos_sim diagonal is used by the reference, so no all-gather).

Per core the work is two reductions over every passage row x (128 irr + 16
rel per sample, D=1024):
    raw[b,r] = sum_d src[b,d] * x[b,r,d]
    ss[b,r]  = sum_d x[b,r,d]^2
The v1 kernel did both on DVE/ACT (1 elem/lane/cycle reduces) and was
compute-bound at ~115-127us per engine. v2 moves both reductions to the
TensorEngine, which ingests 128 rows/cycle (256 with fp8 DoubleRow):

  * Host quantizes everything to fp8e4 (e4m3) and lays the passages out
    d-major: column n = one passage row, partition p = d (chunked).
    DMA drops to ~9.4 MB/core (~26us).
  * raw: matmul(lhsT=srcT[128,2,64], rhs=X[128,2,512]) accumulated over
    4 k-pair-groups in PSUM partitions 0:64 (DoubleRow, K=256/matmul).
  * ss: squares sq = x*x computed elementwise (layout-agnostic) split
    across DVE / ACT / GpSimd, then matmul(lhsT=ones[128,2,64], rhs=sq)
    into PSUM partitions 64:128 of the same bank -> one [128,512] ACT
    copy extracts raw AND ss together (cast to bf16) -> DMA to DRAM.
  * Host picks the per-sample diagonal blocks (column b*144+r of psum row
    b), normalizes with |q_src| (host, exact over the fp8 values) and
    sqrt(ss), and does exp/log/mean in float64 (66K values).
  * src/tgt stats for the loss_main diagonal stay fp32 on device (exact).

fp8 error budget: dot noise ~2.5% of a sigma=32 dot -> ~8e-4 absolute on
each cosine sim; after exp/log/mean the loss error is ~1e-4 relative,
200x inside the 2e-2 gate.
"""

import numpy as np
import ml_dtypes

import concourse.bass as bass
import concourse.mybir as mybir
import concourse.tile as tile
from concourse.bass_utils import run_bass_kernel_spmd

F32 = mybir.dt.float32
BF16 = mybir.dt.bfloat16
F8 = mybir.dt.float8e4
ALPHA = 0.8
B, D, P_REL, N_IRR = 512, 1024, 16, 128
NCORES = 8
BL = B // NCORES          # 64 samples per core
R = N_IRR + P_REL         # 144 passage rows per sample
NCOL = BL * R             # 9216 data columns per core
NT = 4                    # k pair-groups (256 contraction each)
# half-size first/last blocks shrink pipeline fill and drain tails
BLOCK_SIZES = (512, 1024, 1024, 1024, 1024, 1024, 1024, 1024, 1024, 512)
NBLK = len(BLOCK_SIZES)
JT = 512                  # columns per psum tile
NPE4 = ml_dtypes.float8_e4m3

USE_DOUBLE_ROW = True
# squares: each engine squares a column range of all 8 k-rows in one
# 3D-AP instruction; fractions from measured rates (ACT ~0.76 ns/elem,
# DVE fp8 ~2.1, GpSimd ~2.65), ACT/DVE also carry the extraction copies.
SQ_COLS = {1024: (608, 232, 184), 512: (304, 116, 92)}  # (act, dve, gpsimd)


def _split_excess_waits(nc, max_waits=1):
    """This container's walrus rejects instructions carrying more than
    `max_waits` SyncWaits (the TileContext tail drain accumulates several).
    Splice NOPs on the same engine, each carrying a chunk of the waits."""
    import concourse.mybir as mb

    for bb in nc.main_func.blocks:
        while True:
            insts = list(bb.instructions)
            tgt_idx = None
            for i, ins in enumerate(insts):
                si = ins.sync_info
                if si and si.on_wait and len(si.on_wait) > max_waits:
                    tgt_idx = i
                    break
            if tgt_idx is None:
                break
            ins = insts[tgt_idx]
            w = list(ins.sync_info.on_wait)
            keep, extra = w[:max_waits], w[max_waits:]
            nops = []
            for j in range(0, len(extra), max_waits):
                chunk = extra[j : j + max_waits]
                bnop = nc.engines[ins.engine].nop(nofuse=True)
                nop_inst = None
                for bb2 in nc.main_func.blocks:
                    l2 = list(bb2.instructions)
                    for k, cand in enumerate(l2):
                        if cand.name == bnop.ins.name:
                            nop_inst = cand
                            del l2[k]
                            bb2.instructions = l2
                            break
                    if nop_inst is not None:
                        break
                assert nop_inst is not None
                nop_inst.sync_info = mb.SyncInfo(on_wait=chunk, on_update=[])
                nops.append(nop_inst)
            ins.sync_info = mb.SyncInfo(on_wait=keep, on_update=ins.sync_info.on_update)
            insts = list(bb.instructions)
            tgt_idx = next(i for i, x in enumerate(insts) if x.name == ins.name)
            bb.instructions = insts[:tgt_idx] + nops + insts[tgt_idx:]


def _build_program():
    nc = bass.Bass()
    DR = mybir.MatmulPerfMode.DoubleRow if USE_DOUBLE_ROW else None
    xt = nc.dram_tensor("xt", [128, 8 * NCOL], F8, kind="ExternalInput")
    srcw = nc.dram_tensor("srcw", [128, NT * 2 * BL], F8, kind="ExternalInput")
    ones8 = nc.dram_tensor("ones8", [128, 2 * BL], F8, kind="ExternalInput")
    src32 = nc.dram_tensor("src32", [BL, D], F32, kind="ExternalInput")
    tgt32 = nc.dram_tensor("tgt32", [BL, D], F32, kind="ExternalInput")
    res = nc.dram_tensor("res", [65, NCOL], BF16, kind="ExternalOutput")
    stats = nc.dram_tensor("stats", [BL, 4], F32, kind="ExternalOutput")

    Square = mybir.ActivationFunctionType.Square

    with tile.TileContext(nc) as tc:
        with (
            tc.tile_pool(name="persist", bufs=1) as persist,
            tc.tile_pool(name="blocks", bufs=6) as blocks,
            tc.tile_pool(name="sqpool", bufs=6) as sqpool,
            tc.tile_pool(name="work", bufs=2) as work,
            tc.tile_pool(name="psum", bufs=2, space="PSUM") as psum,
            tc.tile_pool(name="psum2", bufs=2, space="PSUM") as psum2,
        ):
            res_sb = persist.tile([65, NCOL], BF16)

            # --- main streaming loop over column blocks ---
            # ss-matmuls for block n are emitted after block n+1's raw
            # matmuls so the PE never stalls waiting for square production.
            col_off = [0]
            for cb in BLOCK_SIZES:
                col_off.append(col_off[-1] + cb)
            sqs, rawps = {}, {}

            ssps = {}

            def emit_ss(n):
                cb = BLOCK_SIZES[n]
                nj = cb // JT
                sq_n = sqs.pop(n)
                # uniform max-size psum tiles keep the bank budget at 8
                ssp = psum2.tile([BL, 1024], F32, tag="ssp", name=f"ssf_{n}")
                for j in range(nj):
                    for t in range(NT):
                        cols = slice(j * JT, (j + 1) * JT)
                        nc.tensor.matmul(
                            ssp[:, j * JT : (j + 1) * JT],
                            lhsT=ones_t[:, :, :],
                            rhs=sq_n[:, 2 * t : 2 * t + 2, cols],
                            start=(t == 0),
                            stop=(t == NT - 1),
                            perf_mode=DR,
                            skip_group_check=True,
                        )
                ssps[n] = ssp

            def emit_extract(n):
                cb = BLOCK_SIZES[n]
                c0 = col_off[n]
                rawp = rawps.pop(n)
                ssp = ssps.pop(n)
                # ss psum rows are 64 identical copies; keep just row 64
                nc.scalar.copy(res_sb[0:BL, c0 : c0 + cb], rawp[:, 0:cb])
                nc.vector.tensor_copy(res_sb[BL : BL + 1, c0 : c0 + cb], ssp[0:1, 0:cb])
                nc.sync.dma_start(
                    out=res[:, c0 : c0 + cb], in_=res_sb[:, c0 : c0 + cb]
                )

            for blk in range(NBLK):
                cb = BLOCK_SIZES[blk]
                nj = cb // JT
                xin = blocks.tile([128, 8, cb], F8, tag=f"xin{cb}")
                # block 0 rides the otherwise-idle ACT HWDGE queue so two
                # input blocks land in parallel during the ramp
                dma_eng = nc.scalar if blk == 0 else nc.sync
                dma_eng.dma_start(
                    out=xin[:, :, :],
                    in_=xt[:, 8 * col_off[blk] : 8 * col_off[blk + 1]],
                )
                if blk == 0:
                    # small persistent loads on the ACT queue (idle during
                    # ramp) so the sync queue stays a pure xt stream
                    srcw_t = persist.tile([128, NT, 2, BL], F8)
                    nc.scalar.dma_start(out=srcw_t[:, :, :, :], in_=srcw[:, :])
                    ones_t = persist.tile([128, 2, BL], F8)
                    nc.scalar.dma_start(out=ones_t[:, :, :], in_=ones8[:, :])
                    src_f = persist.tile([BL, D], F32)
                    nc.scalar.dma_start(out=src_f[:, :], in_=src32[:, :])
                    tgt_f = persist.tile([BL, D], F32)
                    nc.scalar.dma_start(out=tgt_f[:, :], in_=tgt32[:, :])
                if blk == 1:
                    # fp32 src/tgt stats for the exact diag term
                    stats_sb = persist.tile([BL, 4], F32)
                    dummy_act = persist.tile([BL, 1], F32)
                    prod_st = work.tile([BL, D], F32, tag="prodst")
                    nc.vector.tensor_mul(prod_st[:, :], src_f[:, :], tgt_f[:, :])
                    nc.vector.tensor_reduce(
                        stats_sb[:, 0:1], prod_st[:, :], axis=mybir.AxisListType.X,
                        op=mybir.AluOpType.add,
                    )
                    nc.scalar.activation(
                        dummy_act[:, 0:1].broadcast_to((BL, D)), src_f[:, :],
                        Square, accum_out=stats_sb[:, 1:2],
                    )
                    nc.scalar.activation(
                        dummy_act[:, 0:1].broadcast_to((BL, D)), tgt_f[:, :],
                        Square, accum_out=stats_sb[:, 2:3],
                    )
                    nc.vector.memset(stats_sb[:, 3:4], 0.0)
                    nc.sync.dma_start(out=stats[:, :], in_=stats_sb[:, :])
                # squares, column-split across engines
                sq = sqpool.tile([128, 8, cb], F8, tag=f"sq{cb}")
                ca, cv, cg = SQ_COLS[cb]
                nc.scalar.activation(
                    sq[:, :, 0:ca], xin[:, :, 0:ca], Square
                )
                nc.vector.tensor_mul(
                    sq[:, :, ca : ca + cv],
                    xin[:, :, ca : ca + cv],
                    xin[:, :, ca : ca + cv],
                )
                nc.gpsimd.tensor_mul(
                    sq[:, :, ca + cv : cb],
                    xin[:, :, ca + cv : cb],
                    xin[:, :, ca + cv : cb],
                )
                sqs[blk] = sq

                # raw: t-outer so each srcw_t weight load covers nj matmuls
                rawp = psum.tile([BL, 1024], F32, tag="rawp", name=f"ptf_{blk}")
                for t in range(NT):
                    for j in range(nj):
                        cols = slice(j * JT, (j + 1) * JT)
                        nc.tensor.matmul(
                            rawp[:, j * JT : (j + 1) * JT],
                            lhsT=srcw_t[:, t, :, :],
                            rhs=xin[:, 2 * t : 2 * t + 2, cols],
                            start=(t == 0),
                            stop=(t == NT - 1),
                            perf_mode=DR,
                            skip_group_check=True,
                        )
                rawps[blk] = rawp
                if blk > 0:
                    emit_ss(blk - 1)
                if blk > 1:
                    emit_extract(blk - 2)
            emit_ss(NBLK - 1)
            emit_extract(NBLK - 2)
            emit_extract(NBLK - 1)

    _split_excess_waits(nc, max_waits=1)
    return nc


_NC_CACHE = None


def _get_nc():
    global _NC_CACHE
    if _NC_CACHE is None:
        _NC_CACHE = _build_program()
    return _NC_CACHE


def _run_device(in_maps, trace=False, **kw):
    nc = _get_nc()
    return run_bass_kernel_spmd(
        nc, in_maps, core_ids=list(range(NCORES)), trace=trace, **kw
    )


def make_in_maps(embeddings_src, embeddings_target, relevant_passage, irrelevant_passage):
    src = np.asarray(embeddings_src, dtype=np.float32)
    tgt = np.asarray(embeddings_target, dtype=np.float32)
    rel = np.asarray(relevant_passage, dtype=np.float32)
    irr = np.asarray(irrelevant_passage, dtype=np.float32)

    # fp8 quantization (once, full batch)
    src8 = src.astype(NPE4)
    x8 = np.concatenate(
        [irr.astype(NPE4), rel.astype(NPE4)], axis=1
    )  # [B, R, D] rows: 128 irr then 16 rel

    ones_block = np.ones((128, 2 * BL), dtype=NPE4)

    in_maps = []
    for c in range(NCORES):
        sl = slice(c * BL, (c + 1) * BL)
        Xc = x8[sl].reshape(NCOL, D)  # [9216, 1024] b-major rows
        # per block: xt[p, 8*c0 + (t*2+ko)*cb + col] = Xc[c0+col, 256t+128ko+p]
        parts = []
        c0 = 0
        for cb in BLOCK_SIZES:
            parts.append(
                np.ascontiguousarray(
                    Xc[c0 : c0 + cb].reshape(cb, NT, 2, 128).transpose(3, 1, 2, 0)
                ).reshape(128, 8 * cb)
            )
            c0 += cb
        xt_c = np.concatenate(parts, axis=1)
        # srcw[p, t, ko, m] = src8[m, 256t + 128ko + p]
        srcw_c = np.ascontiguousarray(
            src8[sl].reshape(BL, NT, 2, 128).transpose(3, 1, 2, 0)
        ).reshape(128, NT * 2 * BL)
        in_maps.append(
            {
                "xt": xt_c,
                "srcw": srcw_c,
                "ones8": ones_block,
                "src32": np.ascontiguousarray(src[sl]),
                "tgt32": np.ascontiguousarray(tgt[sl]),
            }
        )
    return in_maps


def finish_on_host(core_outs, src8_norms):
    """core_outs: list of (res [128, NCOL] bf16, stats [BL, 4] f32).
    src8_norms: [B] float64 norms of the fp8-quantized src rows."""
    raw = np.empty((B, R), np.float64)
    ss = np.empty((B, R), np.float64)
    st_dot = np.empty((B,), np.float64)
    ss_src = np.empty((B,), np.float64)
    ss_tgt = np.empty((B,), np.float64)
    for c, (res_c, stats_c) in enumerate(core_outs):
        res_c = np.asarray(res_c).astype(np.float64)
        stats_c = np.asarray(stats_c).astype(np.float64)
        for b in range(BL):
            gb = c * BL + b
            cols = slice(b * R, (b + 1) * R)
            raw[gb] = res_c[b, cols]
            ss[gb] = res_c[BL, cols]
        bsl = slice(c * BL, (c + 1) * BL)
        st_dot[bsl] = stats_c[:, 0]
        ss_src[bsl] = stats_c[:, 1]
        ss_tgt[bsl] = stats_c[:, 2]

    diag = st_dot / np.clip(np.sqrt(ss_src * ss_tgt), 1e-12, None)
    sims = raw / np.clip(
        src8_norms[:, None] * np.sqrt(np.clip(ss, 1e-24, None)), 1e-12, None
    )
    neg_sims = sims[:, :N_IRR]
    pos_sims = sims[:, N_IRR:]
    pos_score = 1.0 + np.exp(pos_sims).sum(axis=1)
    neg_score = np.exp(neg_sims).sum(axis=1)
    loss_pos = np.log(pos_score)
    loss_neg = np.log(pos_score + neg_score)
    loss = np.mean(-(ALPHA * diag + (1.0 - ALPHA) * (loss_pos - loss_neg)))
    return np.float32(loss)


def kernel(embeddings_src, embeddings_target, relevant_passage, irrelevant_passage):
    in_maps = make_in_maps(
        embeddings_src, embeddings_target, relevant_passage, irrelevant_passage
    )
    src8 = np.asarray(embeddings_src, dtype=np.float32).astype(NPE4)
    src8_norms = np.sqrt(
        np.square(src8.astype(np.float64)).sum(axis=1)
    )
    res = _run_device(in_maps)
    outs = [
        (res.results[c]["res"], res.results[c]["stats"]) for c in range(NCORES)
    ]
    return np.asarray(finish_on_host(outs, src8_norms), dtype=np.float32)


# revision 9
# speedup vs baseline: 1.1084x; 1.1084x over previous
"""Trainium2 Bass kernel for nn_ContrastiveLoss — PE-centric v2.

Strategy
--------
Data-parallel over batch B=512: 8 cores x 64 samples, fully independent
(only the cos_sim diagonal is used by the reference, so no all-gather).

Per core the work is two reductions over every passage row x (128 irr + 16
rel per sample, D=1024):
    raw[b,r] = sum_d src[b,d] * x[b,r,d]
    ss[b,r]  = sum_d x[b,r,d]^2
The v1 kernel did both on DVE/ACT (1 elem/lane/cycle reduces) and was
compute-bound at ~115-127us per engine. v2 moves both reductions to the
TensorEngine, which ingests 128 rows/cycle (256 with fp8 DoubleRow):

  * Host quantizes everything to fp8e4 (e4m3) and lays the passages out
    d-major: column n = one passage row, partition p = d (chunked).
    DMA drops to ~9.4 MB/core (~26us).
  * raw: matmul(lhsT=srcT[128,2,64], rhs=X[128,2,512]) accumulated over
    4 k-pair-groups in PSUM partitions 0:64 (DoubleRow, K=256/matmul).
  * ss: squares sq = x*x computed elementwise (layout-agnostic) split
    across DVE / ACT / GpSimd, then matmul(lhsT=ones[128,2,64], rhs=sq)
    into PSUM partitions 64:128 of the same bank -> one [128,512] ACT
    copy extracts raw AND ss together (cast to bf16) -> DMA to DRAM.
  * Host picks the per-sample diagonal blocks (column b*144+r of psum row
    b), normalizes with |q_src| (host, exact over the fp8 values) and
    sqrt(ss), and does exp/log/mean in float64 (66K values).
  * src/tgt stats for the loss_main diagonal stay fp32 on device (exact).

fp8 error budget: dot noise ~2.5% of a sigma=32 dot -> ~8e-4 absolute on
each cosine sim; after exp/log/mean the loss error is ~1e-4 relative,
200x inside the 2e-2 gate.
"""

import numpy as np
import ml_dtypes

import concourse.bass as bass
import concourse.mybir as mybir
import concourse.tile as tile
from concourse.bass_utils import run_bass_kernel_spmd

F32 = mybir.dt.float32
BF16 = mybir.dt.bfloat16
F8 = mybir.dt.float8e4
ALPHA = 0.8
B, D, P_REL, N_IRR = 512, 1024, 16, 128
NCORES = 8
BL = B // NCORES          # 64 samples per core
R = N_IRR + P_REL         # 144 passage rows per sample
NCOL = BL * R             # 9216 data columns per core
NT = 4                    # k pair-groups (256 contraction each)
# half-size first/last blocks shrink pipeline fill and drain tails
BLOCK_SIZES = (512, 1024, 1024, 1024, 1024, 1024, 1024, 1024, 1024, 512)
NBLK = len(BLOCK_SIZES)
JT = 512                  # columns per psum tile
NPE4 = ml_dtypes.float8_e4m3

USE_DOUBLE_ROW = True
# squares: each engine squares a column range of all 8 k-rows in one
# 3D-AP instruction; fractions from measured rates (ACT ~0.76 ns/elem,
# DVE fp8 ~2.1, GpSimd ~2.65), ACT/DVE also carry the extraction copies.
SQ_COLS = {1024: (608, 232, 184), 512: (304, 116, 92)}  # (act, dve, gpsimd)


def _split_excess_waits(nc, max_waits=1):
    """This container's walrus rejects instructions carrying more than
    `max_waits` SyncWaits (the TileContext tail drain accumulates several).
    Splice NOPs on the same engine, each carrying a chunk of the waits."""
    import concourse.mybir as mb

    for bb in nc.main_func.blocks:
        while True:
            insts = list(bb.instructions)
            tgt_idx = None
            for i, ins in enumerate(insts):
                si = ins.sync_info
                if si and si.on_wait and len(si.on_wait) > max_waits:
                    tgt_idx = i
                    break
            if tgt_idx is None:
                break
            ins = insts[tgt_idx]
            w = list(ins.sync_info.on_wait)
            keep, extra = w[:max_waits], w[max_waits:]
            nops = []
            for j in range(0, len(extra), max_waits):
                chunk = extra[j : j + max_waits]
                bnop = nc.engines[ins.engine].nop(nofuse=True)
                nop_inst = None
                for bb2 in nc.main_func.blocks:
                    l2 = list(bb2.instructions)
                    for k, cand in enumerate(l2):
                        if cand.name == bnop.ins.name:
                            nop_inst = cand
                            del l2[k]
                            bb2.instructions = l2
                            break
                    if nop_inst is not None:
                        break
                assert nop_inst is not None
                nop_inst.sync_info = mb.SyncInfo(on_wait=chunk, on_update=[])
                nops.append(nop_inst)
            ins.sync_info = mb.SyncInfo(on_wait=keep, on_update=ins.sync_info.on_update)
            insts = list(bb.instructions)
            tgt_idx = next(i for i, x in enumerate(insts) if x.name == ins.name)
            bb.instructions = insts[:tgt_idx] + nops + insts[tgt_idx:]


def _build_program():
    nc = bass.Bass()
    DR = mybir.MatmulPerfMode.DoubleRow if USE_DOUBLE_ROW else None
    xt = nc.dram_tensor("xt", [128, 8 * NCOL], F8, kind="ExternalInput")
    srcw = nc.dram_tensor("srcw", [128, NT * 2 * BL], F8, kind="ExternalInput")
    ones8 = nc.dram_tensor("ones8", [128, 2 * BL], F8, kind="ExternalInput")
    src32 = nc.dram_tensor("src32", [BL, D], F32, kind="ExternalInput")
    tgt32 = nc.dram_tensor("tgt32", [BL, D], F32, kind="ExternalInput")
    res = nc.dram_tensor("res", [65, NCOL], BF16, kind="ExternalOutput")
    stats = nc.dram_tensor("stats", [BL, 4], F32, kind="ExternalOutput")

    Square = mybir.ActivationFunctionType.Square

    with tile.TileContext(nc) as tc:
        with (
            tc.tile_pool(name="persist", bufs=1) as persist,
            tc.tile_pool(name="blocks", bufs=6) as blocks,
            tc.tile_pool(name="sqpool", bufs=6) as sqpool,
            tc.tile_pool(name="work", bufs=2) as work,
            tc.tile_pool(name="psum", bufs=2, space="PSUM") as psum,
            tc.tile_pool(name="psum2", bufs=2, space="PSUM") as psum2,
        ):
            res_sb = persist.tile([65, NCOL], BF16)

            # --- main streaming loop over column blocks ---
            # ss-matmuls for block n are emitted after block n+1's raw
            # matmuls so the PE never stalls waiting for square production.
            col_off = [0]
            for cb in BLOCK_SIZES:
                col_off.append(col_off[-1] + cb)
            sqs, rawps = {}, {}

            ssps = {}

            def emit_ss(n):
                cb = BLOCK_SIZES[n]
                nj = cb // JT
                sq_n = sqs.pop(n)
                # uniform max-size psum tiles keep the bank budget at 8
                ssp = psum2.tile([BL, 1024], F32, tag="ssp", name=f"ssf_{n}")
                for j in range(nj):
                    for t in range(NT):
                        cols = slice(j * JT, (j + 1) * JT)
                        nc.tensor.matmul(
                            ssp[:, j * JT : (j + 1) * JT],
                            lhsT=ones_t[:, :, :],
                            rhs=sq_n[:, 2 * t : 2 * t + 2, cols],
                            start=(t == 0),
                            stop=(t == NT - 1),
                            perf_mode=DR,
                            skip_group_check=True,
                        )
                ssps[n] = ssp

            def emit_extract(n):
                cb = BLOCK_SIZES[n]
                c0 = col_off[n]
                rawp = rawps.pop(n)
                ssp = ssps.pop(n)
                # ss psum rows are 64 identical copies; keep just row 64
                nc.scalar.copy(res_sb[0:BL, c0 : c0 + cb], rawp[:, 0:cb])
                nc.vector.tensor_copy(res_sb[BL : BL + 1, c0 : c0 + cb], ssp[0:1, 0:cb])
                nc.sync.dma_start(
                    out=res[:, c0 : c0 + cb], in_=res_sb[:, c0 : c0 + cb]
                )

            for blk in range(NBLK):
                cb = BLOCK_SIZES[blk]
                nj = cb // JT
                xin = blocks.tile([128, 8, cb], F8, tag=f"xin{cb}")
                nc.sync.dma_start(
                    out=xin[:, :, :],
                    in_=xt[:, 8 * col_off[blk] : 8 * col_off[blk + 1]],
                )
                if blk == 0:
                    # small persistent loads, emitted after the first xt
                    # block so streaming starts immediately
                    srcw_t = persist.tile([128, NT, 2, BL], F8)
                    nc.sync.dma_start(out=srcw_t[:, :, :, :], in_=srcw[:, :])
                    ones_t = persist.tile([128, 2, BL], F8)
                    nc.sync.dma_start(out=ones_t[:, :, :], in_=ones8[:, :])
                    src_f = persist.tile([BL, D], F32)
                    nc.sync.dma_start(out=src_f[:, :], in_=src32[:, :])
                    tgt_f = persist.tile([BL, D], F32)
                    nc.sync.dma_start(out=tgt_f[:, :], in_=tgt32[:, :])
                if blk == 1:
                    # fp32 src/tgt stats for the exact diag term
                    stats_sb = persist.tile([BL, 4], F32)
                    dummy_act = persist.tile([BL, 1], F32)
                    prod_st = work.tile([BL, D], F32, tag="prodst")
                    nc.vector.tensor_mul(prod_st[:, :], src_f[:, :], tgt_f[:, :])
                    nc.vector.tensor_reduce(
                        stats_sb[:, 0:1], prod_st[:, :], axis=mybir.AxisListType.X,
                        op=mybir.AluOpType.add,
                    )
                    nc.scalar.activation(
                        dummy_act[:, 0:1].broadcast_to((BL, D)), src_f[:, :],
                        Square, accum_out=stats_sb[:, 1:2],
                    )
                    nc.scalar.activation(
                        dummy_act[:, 0:1].broadcast_to((BL, D)), tgt_f[:, :],
                        Square, accum_out=stats_sb[:, 2:3],
                    )
                    nc.vector.memset(stats_sb[:, 3:4], 0.0)
                    nc.sync.dma_start(out=stats[:, :], in_=stats_sb[:, :])
                # squares, column-split across engines
                sq = sqpool.tile([128, 8, cb], F8, tag=f"sq{cb}")
                ca, cv, cg = SQ_COLS[cb]
                nc.scalar.activation(
                    sq[:, :, 0:ca], xin[:, :, 0:ca], Square
                )
                nc.vector.tensor_mul(
                    sq[:, :, ca : ca + cv],
                    xin[:, :, ca : ca + cv],
                    xin[:, :, ca : ca + cv],
                )
                nc.gpsimd.tensor_mul(
                    sq[:, :, ca + cv : cb],
                    xin[:, :, ca + cv : cb],
                    xin[:, :, ca + cv : cb],
                )
                sqs[blk] = sq

                # raw: t-outer so each srcw_t weight load covers nj matmuls
                rawp = psum.tile([BL, 1024], F32, tag="rawp", name=f"ptf_{blk}")
                for t in range(NT):
                    for j in range(nj):
                        cols = slice(j * JT, (j + 1) * JT)
                        nc.tensor.matmul(
                            rawp[:, j * JT : (j + 1) * JT],
                            lhsT=srcw_t[:, t, :, :],
                            rhs=xin[:, 2 * t : 2 * t + 2, cols],
                            start=(t == 0),
                            stop=(t == NT - 1),
                            perf_mode=DR,
                            skip_group_check=True,
                        )
                rawps[blk] = rawp
                if blk > 0:
                    emit_ss(blk - 1)
                if blk > 1:
                    emit_extract(blk - 2)
            emit_ss(NBLK - 1)
            emit_extract(NBLK - 2)
            emit_extract(NBLK - 1)

    _split_excess_waits(nc, max_waits=1)
    return nc


_NC_CACHE = None


def _get_nc():
    global _NC_CACHE
    if _NC_CACHE is None:
        _NC_CACHE = _build_program()
    return _NC_CACHE


def _run_device(in_maps, trace=False, **kw):
    nc = _get_nc()
    return run_bass_kernel_spmd(
        nc, in_maps, core_ids=list(range(NCORES)), trace=trace, **kw
    )


def make_in_maps(embeddings_src, embeddings_target, relevant_passage, irrelevant_passage):
    src = np.asarray(embeddings_src, dtype=np.float32)
    tgt = np.asarray(embeddings_target, dtype=np.float32)
    rel = np.asarray(relevant_passage, dtype=np.float32)
    irr = np.asarray(irrelevant_passage, dtype=np.float32)

    # fp8 quantization (once, full batch)
    src8 = src.astype(NPE4)
    x8 = np.concatenate(
        [irr.astype(NPE4), rel.astype(NPE4)], axis=1
    )  # [B, R, D] rows: 128 irr then 16 rel

    ones_block = np.ones((128, 2 * BL), dtype=NPE4)

    in_maps = []
    for c in range(NCORES):
        sl = slice(c * BL, (c + 1) * BL)
        Xc = x8[sl].reshape(NCOL, D)  # [9216, 1024] b-major rows
        # per block: xt[p, 8*c0 + (t*2+ko)*cb + col] = Xc[c0+col, 256t+128ko+p]
        parts = []
        c0 = 0
        for cb in BLOCK_SIZES:
            parts.append(
                np.ascontiguousarray(
                    Xc[c0 : c0 + cb].reshape(cb, NT, 2, 128).transpose(3, 1, 2, 0)
                ).reshape(128, 8 * cb)
            )
            c0 += cb
        xt_c = np.concatenate(parts, axis=1)
        # srcw[p, t, ko, m] = src8[m, 256t + 128ko + p]
        srcw_c = np.ascontiguousarray(
            src8[sl].reshape(BL, NT, 2, 128).transpose(3, 1, 2, 0)
        ).reshape(128, NT * 2 * BL)
        in_maps.append(
            {
                "xt": xt_c,
                "srcw": srcw_c,
                "ones8": ones_block,
                "src32": np.ascontiguousarray(src[sl]),
                "tgt32": np.ascontiguousarray(tgt[sl]),
            }
        )
    return in_maps


def finish_on_host(core_outs, src8_norms):
    """core_outs: list of (res [128, NCOL] bf16, stats [BL, 4] f32).
    src8_norms: [B] float64 norms of the fp8-quantized src rows."""
    raw = np.empty((B, R), np.float64)
    ss = np.empty((B, R), np.float64)
    st_dot = np.empty((B,), np.float64)
    ss_src = np.empty((B,), np.float64)
    ss_tgt = np.empty((B,), np.float64)
    for c, (res_c, stats_c) in enumerate(core_outs):
        res_c = np.asarray(res_c).astype(np.float64)
        stats_c = np.asarray(stats_c).astype(np.float64)
        for b in range(BL):
            gb = c * BL + b
            cols = slice(b * R, (b + 1) * R)
            raw[gb] = res_c[b, cols]
            ss[gb] = res_c[BL, cols]
        bsl = slice(c * BL, (c + 1) * BL)
        st_dot[bsl] = stats_c[:, 0]
        ss_src[bsl] = stats_c[:, 1]
        ss_tgt[bsl] = stats_c[:, 2]

    diag = st_dot / np.clip(np.sqrt(ss_src * ss_tgt), 1e-12, None)
    sims = raw / np.clip(
        src8_norms[:, None] * np.sqrt(np.clip(ss, 1e-24, None)), 1e-12, None
    )
    neg_sims = sims[:, :N_IRR]
    pos_sims = sims[:, N_IRR:]
    pos_score = 1.0 + np.exp(pos_sims).sum(axis=1)
    neg_score = np.exp(neg_sims).sum(axis=1)
    loss_pos = np.log(pos_score)
    loss_neg = np.log(pos_score + neg_score)
    loss = np.mean(-(ALPHA * diag + (1.0 - ALPHA) * (loss_pos - loss_neg)))
    return np.float32(loss)


def kernel(embeddings_src, embeddings_target, relevant_passage, irrelevant_passage):
    in_maps = make_in_maps(
        embeddings_src, embeddings_target, relevant_passage, irrelevant_passage
    )
    src8 = np.asarray(embeddings_src, dtype=np.float32).astype(NPE4)
    src8_norms = np.sqrt(
        np.square(src8.astype(np.float64)).sum(axis=1)
    )
    res = _run_device(in_maps)
    outs = [
        (res.results[c]["res"], res.results[c]["stats"]) for c in range(NCORES)
    ]
    return np.asarray(finish_on_host(outs, src8_norms), dtype=np.float32)
